# revision 22
# baseline (speedup 1.0000x reference)
"""Two-layer GAT (PyG-style, eval mode) on 8 Trainium2 NeuronCores.

Sharding: edges partitioned by destination-node range (6250 nodes/core).
Each core fully owns the segment-softmax + aggregation for its dst range.

v4 layout (vs v2 baseline):
- Per-core PRIVATE h1 table: host hands each core its referenced x rows
  pre-transposed (x_refT); device computes h1 rows once per unique src
  node (~44k rows vs ~119k edges), packs [h1'(512)|asrc(8)|pad] into
  1280B bf16 rows in local DRAM.  No x AllGather at all.
- L1 edge phase gathers bf16 h1 rows directly -> the message product
  M = h1 * alpha runs in DVE fast mode on SBUF bf16 operands (no per-
  edge PSUM crossing), and the per-edge h1 matmul disappears from PE.
- W1 columns reordered (f,h)-major on host so the alpha broadcast has a
  packed last dim (DVE 2x/4x modes); W2 rows permuted to match.
- a_src reaches the edge accumulator via an identity-matmul accumulate
  on PE (free-ish) instead of a DVE add.
- a_dst per dst tile precomputed in one PSUM bank from host-transposed
  x_ownT tiles (kills per-tile DMA-transpose loads in L1).
- h2e attention scalars (asrc2/adst2) folded into the W2 matmul via a
  host-precomputed W2p@att2 vector (watt2) instead of DVE reductions.
- dst-tile loop software-pipelined: stage A (gather/S/S^T/attention) of
  tile t+1 issues before stage B (messages/aggregation/finalize) of
  tile t, so each engine's in-order queue overlaps across tiles.
"""

import os
from contextlib import ExitStack

import numpy as np

# ----------------------------------------------------------------------------
# problem config (hardcoded per contest contract)
# ----------------------------------------------------------------------------
CFG = dict(
    N=50000,       # nodes
    IN=128,        # input feature dim
    HID=64,        # per-head hidden dim
    H1=8,          # layer-1 heads
    NCORES=8,
)

P = 128   # partitions / tile edge
GCH = 7   # edge groups per dma_gather (896 edges; 57*16=912 descs fit the 1024 ring)
MCH = 4   # edge groups per message-product chunk
SCRATCH = 16384  # SWDGE ring bytes/partition (ucode honors only the default 1024 descs)
NCHK = 4  # AllGather chunks (chunk-major gathered-table layout)
ROW = 640  # h1-table row elems (512 h1 + 8 asrc + pad; 1280B, 256B-multiple)


def _cdiv(a, b):
    return (a + b - 1) // b


# ----------------------------------------------------------------------------
# host-side sharding prep (pure layout work: sort, bucket, pad, pack indices)
# ----------------------------------------------------------------------------
def prep_edges(edge_index, cfg):
    """Partition self-loop-augmented edges by dst range across cores; within
    each 128-dst tile split by src half (int16 index limit) and pad each group
    to a multiple of 128 edges.  Tile counts are equalized across cores so a
    single SPMD instruction stream fits every core.

    Returns two index tensors per core: idx16a (position of the edge's src in
    the core's private ref table, for L1) and idx16b (mapped global row, for
    the L2 h2e gathered-table), plus the per-core unique-src (ref) lists.
    """
    N, NC = cfg["N"], cfg["NCORES"]
    NPC = N // NC              # nodes per core
    NT = _cdiv(NPC, P)         # dst tiles per core

    # chunk-major gathered-table layout (matches the chunked AllGathers):
    # chunk ci holds rows [lo_r[ci], hi_r[ci]) of every core, rank-major.
    bounds = [round(i * NT / NCHK) for i in range(NCHK + 1)]
    lo_r = [b * P for b in bounds[:-1]]
    hi_r = [min(b * P, NPC) for b in bounds[1:]]
    rows_c = np.array([hi - lo for lo, hi in zip(lo_r, hi_r)])
    base = np.concatenate([[0], np.cumsum(NC * rows_c)])
    HALF = int(base[NCHK // 2]) if NCHK > 1 else _cdiv(N, 2)

    def map_rows(g):
        c, r = g // NPC, g % NPC
        ci = np.searchsorted(np.array(hi_r), r, side="right")
        return base[ci] + c * rows_c[ci] + (r - np.array(lo_r)[ci])

    src_g = np.concatenate([edge_index[0].astype(np.int64),
                            np.arange(N, dtype=np.int64)])
    dst = np.concatenate([edge_index[1].astype(np.int64),
                          np.arange(N, dtype=np.int64)])
    src = map_rows(src_g)
    order = np.argsort(dst, kind="stable")
    src, src_g, dst = src[order], src_g[order], dst[order]

    groups = [[None] * NT for _ in range(NC)]
    refs = []                  # per core: (lo_mapped_sorted, hi_mapped_sorted)
    core_of = dst // NPC
    core_bounds = np.searchsorted(core_of, np.arange(NC + 1))
    for c in range(NC):
        s0, s1 = core_bounds[c], core_bounds[c + 1]
        d_loc = dst[s0:s1] - c * NPC
        u = np.unique(src[s0:s1])
        refs.append((u[u < HALF], u[u >= HALF]))
        tile_bounds = np.searchsorted(d_loc, np.arange(0, NT * P + 1, P))
        for t in range(NT):
            e0, e1 = s0 + tile_bounds[t], s0 + tile_bounds[t + 1]
            s_t = src[e0:e1]
            slot_t = (dst[e0:e1] - c * NPC - t * P).astype(np.int64)
            lo = s_t < HALF
            groups[c][t] = (s_t[lo], slot_t[lo], s_t[~lo], slot_t[~lo])

    sched = []
    for t in range(NT):
        Lt = max(_cdiv(len(groups[c][t][0]), P) for c in range(NC))
        Ht = max(_cdiv(len(groups[c][t][2]), P) for c in range(NC))
        sched.append((Lt, Ht))

    total_groups = sum(l + h for l, h in sched)
    TI = total_groups * P          # total padded edges per core

    NTRlo = max(_cdiv(len(r[0]), P) for r in refs)
    NTRhi = max(_cdiv(len(r[1]), P) for r in refs)
    assert NTRlo * P < 32768 and NTRhi * P < 32768

    idx16a = np.zeros((NC, 16, TI // 16), dtype=np.int16)
    idx16b = np.zeros((NC, 16, TI // 16), dtype=np.int16)
    dstslot = np.full((NC, P, total_groups), -1.0, dtype=np.float32)

    for c in range(NC):
        lo_ref, hi_ref = refs[c]
        off = 0
        for t in range(NT):
            lo_s, lo_k, hi_s, hi_k = groups[c][t]
            Lt, Ht = sched[t]
            for (ss, kk, ng, ref, hb) in ((lo_s, lo_k, Lt, lo_ref, 0),
                                          (hi_s, hi_k, Ht, hi_ref, HALF)):
                n = ng * P
                if n == 0:
                    continue
                sa = np.zeros(n, dtype=np.int64)       # ref-table position
                sb = np.zeros(n, dtype=np.int64)       # mapped global - hb
                sa[: len(ss)] = np.searchsorted(ref, ss)
                sb[: len(ss)] = ss - hb
                ki = np.full(n, -1.0, dtype=np.float32)
                ki[: len(kk)] = kk
                idx16a[c, :, off // 16: (off + n) // 16] = (
                    sa.reshape(-1, 16).T.astype(np.int16))
                idx16b[c, :, off // 16: (off + n) // 16] = (
                    sb.reshape(-1, 16).T.astype(np.int16))
                g0 = off // P
                dstslot[c, :, g0: g0 + ng] = ki.reshape(-1, P).T
                off += n
        assert off == TI
    idx16a = np.tile(idx16a, (1, 8, 1))
    idx16b = np.tile(idx16b, (1, 8, 1))
    return sched, idx16a, idx16b, dstslot, HALF, refs, NTRlo, NTRhi


# ----------------------------------------------------------------------------
# device kernel
# ----------------------------------------------------------------------------
def build_kernel(cfg, sched, TI, NTRlo, NTRhi, profile=False):
    nocoll = bool(int(os.environ.get("GAT_NOCOLL", "0")))
    import concourse.bacc as bacc
    import concourse.mybir as mybir
    import concourse.tile as tile
    from concourse.masks import make_identity

    N, IN, HID, H1, NC = cfg["N"], cfg["IN"], cfg["HID"], cfg["H1"], cfg["NCORES"]
    NPC = N // NC
    NT = _cdiv(NPC, P)
    NPCP = NT * P                  # padded local rows
    OUT1 = H1 * HID
    TG = TI // P
    NTR = NTRlo + NTRhi
    W2C = _cdiv(OUT1, P)           # W2 row chunks
    f32, bf16 = mybir.dt.float32, mybir.dt.bfloat16
    i16, i32 = mybir.dt.int16, mybir.dt.int32
    AX = mybir.AxisListType
    ALU = mybir.AluOpType
    ACTF = mybir.ActivationFunctionType
    RG = [list(range(NC))]

    nc = bacc.Bacc("TRN2", target_bir_lowering=False, debug=False,
                   num_devices=1 if profile else NC,
                   dynamic_dma_scratch_size=SCRATCH)

    # ---- I/O ----
    xrefT_d = nc.dram_tensor("x_refT", [IN, NTR, P], bf16, kind="ExternalInput")
    xownT_d = nc.dram_tensor("x_ownT", [IN, NT, P], bf16, kind="ExternalInput")
    W1p_d = nc.dram_tensor("W1p", [IN, OUT1], bf16, kind="ExternalInput")
    wsrc_d = nc.dram_tensor("wsrc8", [IN, H1], bf16, kind="ExternalInput")
    wdst_d = nc.dram_tensor("wdst8", [IN, H1], bf16, kind="ExternalInput")
    b1_d = nc.dram_tensor("b1p", [OUT1], f32, kind="ExternalInput")
    W2_d = nc.dram_tensor("W2p", [OUT1, HID], f32, kind="ExternalInput")
    as2_d = nc.dram_tensor("att_src2", [1, HID], f32, kind="ExternalInput")
    ad2_d = nc.dram_tensor("att_dst2", [1, HID], f32, kind="ExternalInput")
    b2_d = nc.dram_tensor("b2", [HID], f32, kind="ExternalInput")
    watt2_d = nc.dram_tensor("watt2", [OUT1, 2], f32, kind="ExternalInput")
    fcw_d = nc.dram_tensor("fc_w", [HID, 1], f32, kind="ExternalInput")
    fcb_d = nc.dram_tensor("fc_b", [1], f32, kind="ExternalInput")
    idxa_d = nc.dram_tensor("idx16a", [P, TI // 16], i16, kind="ExternalInput")
    idxb_d = nc.dram_tensor("idx16b", [P, TI // 16], i16, kind="ExternalInput")
    slot_d = nc.dram_tensor("dstslot", [P, TG], f32, kind="ExternalInput")
    out_d = nc.dram_tensor("out", [NPC, 1], f32, kind="ExternalOutput")

    # ---- internal DRAM ----
    tab = nc.dram_tensor("tab", [NTR * P, ROW], bf16)           # private
    h2e_in = nc.dram_tensor("h2e_in", [NPCP, P], bf16)
    h2e = nc.dram_tensor("h2e", [N, P], bf16, addr_space="Shared")
    ssum_in = nc.dram_tensor("ssum_in", [1, 1], f32)
    ssum = nc.dram_tensor("ssum", [1, 1], f32, addr_space="Shared")

    with tile.TileContext(nc) as tc, ExitStack() as ctx:
        const = ctx.enter_context(tc.tile_pool(name="const", bufs=1))
        sb = ctx.enter_context(tc.tile_pool(name="sb", bufs=2))
        sb3 = ctx.enter_context(tc.tile_pool(name="sb3", bufs=3))
        psA = ctx.enter_context(tc.tile_pool(name="psA", bufs=2, space="PSUM"))
        psC = ctx.enter_context(tc.tile_pool(name="psC", bufs=2, space="PSUM"))

        # ================= constants / weights =================
        idbf = const.tile([P, P], bf16)
        make_identity(nc, idbf[:])
        iota_i = const.tile([P, P], i32)
        nc.gpsimd.iota(iota_i[:], pattern=[[1, P]], base=0,
                       channel_multiplier=0)
        ones_r = const.tile([1, P], f32)
        nc.vector.memset(ones_r[:], 1.0)
        ones_c = const.tile([P, 1], f32)
        nc.vector.memset(ones_c[:], 1.0)
        zpad = const.tile([P, P], bf16)
        nc.vector.memset(zpad[:], 0.0)

        idxa_sb = const.tile([P, TI // 16], i16)
        nc.sync.dma_start(idxa_sb[:], idxa_d.ap())
        idxb_sb = const.tile([P, TI // 16], i16)
        nc.sync.dma_start(idxb_sb[:], idxb_d.ap())
        slot_sb = const.tile([P, TG], f32)
        nc.sync.dma_start(slot_sb[:], slot_d.ap())
        slot_bf = const.tile([P, TG], bf16)
        nc.vector.tensor_copy(slot_bf[:], slot_sb[:])
        iota_bf = const.tile([P, P], bf16)
        nc.vector.tensor_copy(iota_bf[:], iota_i[:])

        w1p_sb = const.tile([P, OUT1], bf16)
        nc.sync.dma_start(w1p_sb[:], W1p_d.ap())
        wsrc_sb = const.tile([P, H1], bf16)
        nc.sync.dma_start(wsrc_sb[:], wsrc_d.ap())
        wdst_sb = const.tile([P, H1], bf16)
        nc.sync.dma_start(wdst_sb[:], wdst_d.ap())
        watt2_f = sb.tile([P, W2C, 2], f32, tag="tmpw5")
        nc.sync.dma_start(
            watt2_f[:], watt2_d.ap().rearrange("(c p) n -> p c n", p=P))
        watt2_sb = const.tile([P, W2C, 2], bf16)
        nc.vector.tensor_copy(watt2_sb[:], watt2_f[:])
        w2b = const.tile([P, W2C, HID], bf16)
        w2f_t = sb.tile([P, W2C, HID], f32, tag="tmpw")
        nc.sync.dma_start(
            w2f_t[:], W2_d.ap().rearrange("(c p) n -> p c n", p=P))
        nc.vector.tensor_copy(w2b[:], w2f_t[:])

        def bcast_row(dram_ap, width, name):
            row = sb.tile([1, width], f32, tag="bcrow")
            nc.sync.dma_start(row[:], dram_ap)
            pt = psC.tile([P, width], f32, tag="stb")
            nc.tensor.matmul(pt[:], lhsT=ones_r[:], rhs=row[:], start=True,
                             stop=True)
            out = const.tile([P, width], f32, tag=name)
            nc.scalar.copy(out[:], pt[:])
            return out

        att2s_bc = bcast_row(as2_d.ap(), HID, "a2s")
        att2d_bc = bcast_row(ad2_d.ap(), HID, "a2d")
        b1_bc = bcast_row(b1_d.ap()[None, :], OUT1, "b1")
        b2_bc = bcast_row(b2_d.ap()[None, :], HID, "b2")
        fcb_bc = bcast_row(fcb_d.ap()[None, :], 1, "fcb")

        fcw_f = sb.tile([HID, 1], f32, tag="tmpw4")
        nc.sync.dma_start(fcw_f[:], fcw_d.ap())
        fcw_sb = const.tile([HID, 1], bf16)
        nc.vector.tensor_copy(fcw_sb[:], fcw_f[:])

        adn_all = const.tile([P, NT * H1], bf16)

        logits = const.tile([P, NT], f32, tag="logits")
        nc.vector.memset(logits[:], -1e30)

        bounds = [round(i * NT / NCHK) for i in range(NCHK + 1)]
        chk_lo = [b * P for b in bounds[:-1]]
        chk_hi = [min(b * P, NPC) for b in bounds[1:]]
        chk_base = [0]
        for lo, hi in zip(chk_lo, chk_hi):
            chk_base.append(chk_base[-1] + NC * (hi - lo))

        lo_tab = tab.ap()[0:NTRlo * P, :]
        hi_tab = tab.ap()[NTRlo * P:NTR * P, :]

        # ================= shared edge-phase machinery =====================
        def edge_phase(layer, tile_cb=None):
            L1 = layer == 1
            idx_sb = idxa_sb if L1 else idxb_sb
            NH = H1 if L1 else 1        # heads
            if L1:
                lo_ap, hi_ap = lo_tab, hi_tab
            else:
                lo_ap = h2e.ap()[0:chk_base[NCHK // 2], :]
                hi_ap = h2e.ap()[chk_base[NCHK // 2]:N, :]
            WE = ROW if L1 else P       # gathered row width
            if not L1:
                # one bulk strided read replaces 49 per-tile
                # DMA-transpose + matmul + copy chains
                adn2_all = sb.tile([P, NT], bf16, tag="adn2")
                nc.sync.dma_start(
                    adn2_all[:],
                    h2e_in.ap()[:, HID + 1:HID + 2].rearrange(
                        "(t p) o -> p (t o)", p=P))
            goffs = []
            _g = 0
            for t in range(NT):
                goffs.append(_g)
                _g += sched[t][0] + sched[t][1]

            def stage_A(t):
                """Gathers + selection matrices + attention logits -> p_all."""
                Lt, Ht = sched[t]
                Kt = Lt + Ht
                goff = goffs[t]
                gbase = t * P

                # --- node-side attention (a_dst per local slot) ---
                if L1:
                    adn = adn_all[:, t * H1:(t + 1) * H1]
                else:
                    adn = adn2_all[:, t:t + 1]

                # --- gathered rows for all Kt groups ---
                X1 = sb.tile([P, Kt, WE], bf16, tag="gath", bufs=3)
                off16 = goff * P // 16
                for g0, gn, half_ap in (
                        [(q, min(GCH, Lt - q), lo_ap)
                         for q in range(0, Lt, GCH)]
                        + [(Lt + q, min(GCH, Ht - q), hi_ap)
                           for q in range(0, Ht, GCH)]):
                    n = gn * P
                    idxs = idx_sb[:, off16 + g0 * P // 16:
                                  off16 + (g0 * P + n) // 16]
                    nc.gpsimd.dma_gather(
                        X1[:, g0: g0 + gn, :], half_ap, idxs,
                        n, n, WE, transpose=False)

                # --- selection matrices for all Kt groups ---
                S_all = sb.tile([P, Kt, P], bf16, tag="S", bufs=3)
                nc.vector.tensor_tensor(
                    S_all[:],
                    iota_bf[:, None, :].to_broadcast([P, Kt, P]),
                    slot_bf[:, goff:goff + Kt, None].to_broadcast([P, Kt, P]),
                    op=ALU.is_equal)

                # --- S^T transposes (batched 8 per PSUM bank, one Act copy
                #     per batch) interleaved with the a_src identity-
                #     accumulate matmuls so the PE never stalls ---
                st_all = sb.tile([P, Kt, P], bf16, tag="st", bufs=3)
                ae_p = psA.tile([P, Kt * NH], f32, tag="ae")
                for b0 in range(0, Kt, 8):
                    bk = min(8, Kt - b0)
                    stb = psC.tile([P, 8, P], bf16, tag="stb")
                    for j in range(b0, b0 + bk):
                        nc.tensor.transpose(stb[:, j - b0, :], S_all[:, j, :],
                                            idbf[:])
                    nc.scalar.copy(st_all[:, b0:b0 + bk, :], stb[:, 0:bk, :])
                    if L1:
                        for j in range(b0, b0 + bk):
                            nc.tensor.matmul(
                                ae_p[:, j * NH:(j + 1) * NH],
                                lhsT=idbf[:],
                                rhs=X1[:, j, OUT1:OUT1 + H1],
                                start=(j == 0), stop=False)
                if L1:
                    for j in range(Kt):
                        nc.tensor.matmul(ae_p[:, j * NH:(j + 1) * NH],
                                         lhsT=st_all[:, j, :], rhs=adn,
                                         start=False, stop=(j == Kt - 1))
                    lr = sb.tile([P, Kt * NH], f32, tag="lr", bufs=3)
                    nc.scalar.activation(lr[:], ae_p[:], ACTF.Prelu,
                                         alpha=0.2)
                else:
                    for j in range(Kt):
                        nc.tensor.matmul(ae_p[:, j:j + 1],
                                         lhsT=st_all[:, j, :],
                                         rhs=adn[:, 0:1],
                                         start=(j == 0), stop=(j == Kt - 1))
                    esum = sb.tile([P, Kt], f32, tag="esum", bufs=3)
                    nc.vector.tensor_tensor(
                        esum[:, :, None], ae_p[:, :, None],
                        X1[:, :, HID:HID + 1], op=ALU.add)
                    lr = sb.tile([P, Kt], f32, tag="lr", bufs=3)
                    nc.scalar.activation(lr[:], esum[:], ACTF.Prelu,
                                         alpha=0.2)
                p_all = sb.tile([P, Kt, NH], bf16, tag="p", bufs=3)
                nc.scalar.activation(
                    p_all[:].rearrange("p k h -> p (k h)"), lr[:], ACTF.Exp)
                return X1, S_all, p_all

            def stage_B(t, X1, S_all, p_all):
                """Messages + segment sums + tile finalize."""
                Lt, Ht = sched[t]
                Kt = Lt + Ht
                gbase = t * P
                rows_t = min(NPC - t * P, P)
                # --- pass 2: messages + segment sums ---
                if L1:
                    z_p = psA.tile([P, OUT1], f32, tag="z")
                    s_p = psA.tile([P, NH], f32, tag="s")
                    for m0 in range(0, Kt, MCH):
                        m1 = min(m0 + MCH, Kt)
                        mk = m1 - m0
                        M_c = sb3.tile([P, MCH, OUT1], bf16, tag="M", bufs=2)
                        nc.vector.tensor_tensor(
                            M_c[:, 0:mk].rearrange(
                                "p k (f h) -> p k f h", h=H1),
                            X1[:, m0:m1, 0:OUT1].rearrange(
                                "p k (f h) -> p k f h", h=H1),
                            p_all[:, m0:m1, None, :].to_broadcast(
                                [P, mk, HID, H1]),
                            op=ALU.mult)
                        for j in range(m0, m1):
                            nc.tensor.matmul(
                                z_p[:], lhsT=S_all[:, j, :],
                                rhs=M_c[:, j - m0, :],
                                start=(j == 0), stop=(j == Kt - 1))
                            nc.tensor.matmul(
                                s_p[:], lhsT=S_all[:, j, :],
                                rhs=p_all[:, j, :],
                                start=(j == 0), stop=(j == Kt - 1))
                else:
                    z_p = psA.tile([P, HID + 1], f32, tag="z")
                    for m0 in range(0, Kt, MCH):
                        m1 = min(m0 + MCH, Kt)
                        mk = m1 - m0
                        M_c = sb3.tile([P, MCH, HID + 1], bf16, tag="M2")
                        nc.vector.tensor_tensor(
                            M_c[:, 0:mk, 0:HID],
                            X1[:, m0:m1, 0:HID],
                            p_all[:, m0:m1, :].to_broadcast([P, mk, HID]),
                            op=ALU.mult)
                        nc.vector.tensor_copy(M_c[:, 0:mk, HID:HID + 1],
                                              p_all[:, m0:m1, :])
                        for j in range(m0, m1):
                            nc.tensor.matmul(
                                z_p[:], lhsT=S_all[:, j, :],
                                rhs=M_c[:, j - m0, :],
                                start=(j == 0), stop=(j == Kt - 1))

                # ---------------- finalize dst tile ----------------
                if L1:
                    s_eps = sb.tile([P, NH], f32, tag="seps")
                    nc.vector.tensor_scalar(s_eps[:], s_p[:], 1e-16, None,
                                            op0=ALU.add)
                    s_inv = sb.tile([P, NH], f32, tag="sinv")
                    nc.vector.reciprocal(s_inv[:], s_eps[:])
                    y = sb.tile([P, OUT1], f32, tag="y")
                    nc.vector.tensor_tensor(
                        y[:].rearrange("p (f h) -> p f h", h=H1),
                        z_p[:].rearrange("p (f h) -> p f h", h=H1),
                        s_inv[:, None, :].to_broadcast([P, HID, H1]),
                        op=ALU.mult)
                    nc.vector.tensor_tensor(y[:], y[:], b1_bc[:], op=ALU.add)
                    WY = OUT1
                else:
                    s_eps = sb.tile([P, 1], f32, tag="seps")
                    nc.vector.tensor_scalar(s_eps[:], z_p[:, HID:HID + 1],
                                            1e-16, None, op0=ALU.add)
                    s_inv = sb.tile([P, 1], f32, tag="sinv")
                    nc.vector.reciprocal(s_inv[:], s_eps[:])
                    y = sb.tile([P, HID], f32, tag="y")
                    nc.vector.tensor_scalar(
                        y[:], z_p[:, 0:HID], s_inv[:], None,
                        op0=ALU.mult)
                    nc.vector.tensor_tensor(y[:], y[:], b2_bc[:], op=ALU.add)
                    WY = HID
                # elu(y) = relu(y) + exp(min(y,0)) - 1
                t0 = sb.tile([P, WY], f32, tag="elu0")
                nc.vector.tensor_scalar_min(t0[:], y[:], 0.0)
                ex = sb.tile([P, WY], f32, tag="elu1")
                nc.scalar.activation(ex[:], t0[:], ACTF.Exp)
                ry = sb.tile([P, WY], f32, tag="elu2")
                nc.scalar.activation(ry[:], y[:], ACTF.Relu)
                x2 = sb.tile([P, WY], bf16, tag="x2")
                nc.vector.scalar_tensor_tensor(
                    x2[:], in0=ex[:], scalar=-1.0, in1=ry[:],
                    op0=ALU.add, op1=ALU.add)

                if L1:
                    # h2 = x2 @ W2 via 4 transposed chunks ((f,h)-permuted W2p)
                    xtb = psC.tile([P, 8, P], bf16, tag="stb")
                    for cix in range(W2C):
                        nc.tensor.transpose(
                            xtb[:, cix, :], x2[:, cix * P:(cix + 1) * P],
                            idbf[:])
                    xts = sb3.tile([P, W2C, P], bf16, tag="xts")
                    nc.scalar.copy(xts[:], xtb[:, 0:W2C, :])
                    h2_p = psA.tile([P, HID + 2], f32, tag="s")
                    for cix in range(W2C):
                        nc.tensor.matmul(h2_p[:, 0:HID], lhsT=xts[:, cix, :],
                                         rhs=w2b[:, cix, :],
                                         start=(cix == 0), stop=False)
                        nc.tensor.matmul(h2_p[:, HID:HID + 2],
                                         lhsT=xts[:, cix, :],
                                         rhs=watt2_sb[:, cix, :],
                                         start=False,
                                         stop=(cix == W2C - 1))
                    h2e_sb = sb.tile([P, P], bf16, tag="h2e")
                    nc.vector.memset(h2e_sb[:, HID + 2:], 0.0)
                    nc.scalar.copy(h2e_sb[:, 0:HID + 2], h2_p[:])
                    nc.sync.dma_start(
                        h2e_in.ap()[gbase:gbase + rows_t, :],
                        h2e_sb[0:rows_t, :])
                else:
                    x2t_p = psC.tile([HID, P], bf16, tag="stb")
                    nc.tensor.transpose(x2t_p[:], x2[:, 0:HID], idbf[:])
                    x2t = sb3.tile([HID, P], bf16, tag="x2t")
                    nc.scalar.copy(x2t[:], x2t_p[:])
                    lg_p = psC.tile([P, 1], f32, tag="stb")
                    nc.tensor.matmul(lg_p[:], lhsT=x2t[:], rhs=fcw_sb[:],
                                     start=True, stop=True)
                    nc.scalar.activation(logits[0:rows_t, t:t + 1],
                                         lg_p[0:rows_t, :], ACTF.Identity,
                                         bias=fcb_bc[0:rows_t, :])
                if tile_cb is not None:
                    tile_cb(t)

            # software pipeline (lookahead-2): stage_A of tiles t+1, t+2
            # issue before stage_B of tile t so each engine's in-order queue
            # overlaps the latency-bound per-tile chains across 3 tiles
            from collections import deque
            pend = deque()
            for t in range(NT):
                pend.append((t,) + stage_A(t))
                if len(pend) > 2:
                    stage_B(*pend.popleft())
            while pend:
                stage_B(*pend.popleft())

        # ================= main (repeatable for timing) =================
        for _rep in range(int(os.environ.get("GAT_REPEAT", "1"))):
            # ---- phase 1a: a_dst per local node (one PSUM bank) ----
            adn_ps = psA.tile([P, NT * H1], f32, tag="ae")
            for t0 in range(0, NT, 4):
                bn = min(4, NT - t0)
                xot = sb3.tile([P, 4, P], bf16, tag="xot")
                nc.sync.dma_start(xot[:, 0:bn, :],
                                  xownT_d.ap()[:, t0:t0 + bn, :])
                for t in range(t0, t0 + bn):
                    nc.tensor.matmul(adn_ps[:, t * H1:(t + 1) * H1],
                                     lhsT=xot[:, t - t0, :], rhs=wdst_sb[:],
                                     start=(t == 0), stop=(t == NT - 1))
            nc.scalar.copy(adn_all[:], adn_ps[:])

            # ---- phase 1b: private h1 table [h1'(512)|asrc(8)|pad] ----
            for r0 in range(0, NTR, 4):
                rn = min(4, NTR - r0)
                xrt = sb3.tile([P, 4, P], bf16, tag="xrt")
                nc.sync.dma_start(xrt[:, 0:rn, :],
                                  xrefT_d.ap()[:, r0:r0 + rn, :])
                rows = sb3.tile([P, 4, ROW], bf16, tag="rows", bufs=2)
                nc.vector.memset(rows[:, :, OUT1 + H1:], 0.0)
                for r in range(r0, r0 + rn):
                    h1p = psA.tile([P, OUT1], f32,
                                   tag=("z" if r % 2 else "s"))
                    nc.tensor.matmul(h1p[:], lhsT=xrt[:, r - r0, :],
                                     rhs=w1p_sb[:], start=True, stop=True)
                    asp = psC.tile([P, H1], f32, tag="stb")
                    nc.tensor.matmul(asp[:], lhsT=xrt[:, r - r0, :],
                                     rhs=wsrc_sb[:], start=True, stop=True)
                    if r % 2:
                        nc.vector.tensor_copy(rows[:, r - r0, 0:OUT1], h1p[:])
                        nc.vector.tensor_copy(
                            rows[:, r - r0, OUT1:OUT1 + H1], asp[:])
                    else:
                        nc.scalar.copy(rows[:, r - r0, 0:OUT1], h1p[:])
                        nc.scalar.copy(rows[:, r - r0, OUT1:OUT1 + H1],
                                       asp[:])
                nc.sync.dma_start(
                    tab.ap()[r0 * P:(r0 + rn) * P, :].rearrange(
                        "(k p) f -> p k f", p=P),
                    rows[:, 0:rn, :])

            # ---- layer 1 (h2e table AllGathered chunk-wise via callback) ----
            if NPCP > NPC:
                nc.sync.dma_start(h2e_in.ap()[NPC:NPCP, :],
                                  zpad[: NPCP - NPC, :])

            def h2e_cb(t):
                for ci in range(NCHK):
                    if t == bounds[ci + 1] - 1:
                        lo, hi = chk_lo[ci], chk_hi[ci]
                        if profile or nocoll:
                            nc.sync.dma_start(
                                h2e.ap()[chk_base[ci]:chk_base[ci] + hi - lo,
                                         :],
                                h2e_in.ap()[lo:hi, :])
                        else:
                            nc.gpsimd.collective_compute(
                                "AllGather", ALU.bypass, replica_groups=RG,
                                ins=[h2e_in.ap()[lo:hi, :].opt()],
                                outs=[h2e.ap()[chk_base[ci]:chk_base[ci + 1],
                                               :].opt()])

            edge_phase(1, tile_cb=h2e_cb)

            # ---- layer 2 ----
            edge_phase(2)

        # ================= softmax over all nodes =================
        ex_all = sb.tile([P, NT], f32, tag="exall")
        nc.scalar.activation(ex_all[:], logits[:], ACTF.Exp)
        part = sb.tile([P, 1], f32, tag="part")
        nc.vector.tensor_reduce(part[:], ex_all[:], axis=AX.X, op=ALU.add)
        tot_p = psC.tile([1, 1], f32, tag="stb")
        nc.tensor.matmul(tot_p[:], lhsT=part[:], rhs=ones_c[:], start=True,
                         stop=True)
        tot_sb = sb.tile([1, 1], f32, tag="tot")
        nc.scalar.copy(tot_sb[:], tot_p[:])
        nc.sync.dma_start(ssum_in.ap(), tot_sb[:])
        if profile or nocoll:
            nc.sync.dma_start(ssum.ap(), ssum_in.ap())
        else:
            nc.gpsimd.collective_compute(
                "AllReduce", ALU.add, replica_groups=RG,
                ins=[ssum_in.ap().opt()], outs=[ssum.ap().opt()])
        gsum = sb.tile([1, 1], f32, tag="gsum")
        nc.sync.dma_start(gsum[:], ssum.ap())
        ginv = sb.tile([1, 1], f32, tag="ginv")
        nc.vector.reciprocal(ginv[:], gsum[:])
        ginv_p = psC.tile([P, 1], f32, tag="stb")
        nc.tensor.matmul(ginv_p[:], lhsT=ones_r[:], rhs=ginv[:], start=True,
                         stop=True)
        ginv_bc = sb.tile([P, 1], f32, tag="ginvbc")
        nc.scalar.copy(ginv_bc[:], ginv_p[:])
        res = sb.tile([P, NT], f32, tag="res")
        nc.vector.tensor_scalar(res[:], ex_all[:], ginv_bc[:], None,
                                op0=ALU.mult)
        full_t = NPC // P
        nc.sync.dma_start(
            out_d.ap()[0:full_t * P, :].rearrange("(t p) o -> p (t o)", p=P),
            res[:, 0:full_t])
        if NPC % P:
            nc.sync.dma_start(out_d.ap()[full_t * P: NPC, :],
                              res[0: NPC % P, full_t:full_t + 1])

    nc.compile()
    return nc


# ----------------------------------------------------------------------------
# entry point
# ----------------------------------------------------------------------------
def build_in_maps(inputs, cfg):
    import ml_dtypes
    bf16 = ml_dtypes.bfloat16

    sched, idx16a, idx16b, dstslot, HALF, refs, NTRlo, NTRhi = prep_edges(
        np.asarray(inputs["edge_index"]), cfg)
    x = np.asarray(inputs["x"], dtype=np.float32)
    N, NC = cfg["N"], cfg["NCORES"]
    NPC = N // NC
    NT = _cdiv(NPC, P)
    H1, HID, IN = cfg["H1"], cfg["HID"], cfg["IN"]
    OUT1 = H1 * HID
    NTR = NTRlo + NTRhi

    # inverse of prep_edges.map_rows (chunk-major layout)
    bounds = [round(i * NT / NCHK) for i in range(NCHK + 1)]
    lo_r = [b * P for b in bounds[:-1]]
    hi_r = [min(b * P, NPC) for b in bounds[1:]]
    rows_c = np.array([hi - lo for lo, hi in zip(lo_r, hi_r)])
    base = np.concatenate([[0], np.cumsum(NC * rows_c)])
    inv = np.empty(N, dtype=np.int64)
    g = np.arange(N, dtype=np.int64)
    c, r = g // NPC, g % NPC
    ci = np.searchsorted(np.array(hi_r), r, side="right")
    inv[base[ci] + c * rows_c[ci] + (r - np.array(lo_r)[ci])] = g

    # weight prep: (f,h)-major reorder of W1 columns; fold att vectors
    W1 = np.asarray(inputs["W1"], np.float32)
    as1 = np.asarray(inputs["att_src1"], np.float32)
    ad1 = np.asarray(inputs["att_dst1"], np.float32)
    b1 = np.asarray(inputs["b1"], np.float32)
    W2 = np.asarray(inputs["W2"], np.float32)
    order = (np.arange(OUT1).reshape(HID, H1) % H1) * HID + \
        np.arange(OUT1).reshape(HID, H1) // H1
    order = order.reshape(-1)
    W1r = W1.reshape(IN, H1, HID)
    wsrc8 = np.einsum("ihf,hf->ih", W1r, as1)
    wdst8 = np.einsum("ihf,hf->ih", W1r, ad1)
    W1p = np.ascontiguousarray(W1[:, order])
    b1p = np.ascontiguousarray(b1[order])
    W2p = np.ascontiguousarray(W2[order, :])

    as2 = np.asarray(inputs["att_src2"], np.float32)
    ad2 = np.asarray(inputs["att_dst2"], np.float32)
    watt2 = np.stack([W2p @ as2[0], W2p @ ad2[0]], axis=1)

    common = {
        "watt2": np.ascontiguousarray(watt2),
        "W1p": W1p.astype(bf16),
        "wsrc8": np.ascontiguousarray(wsrc8).astype(bf16),
        "wdst8": np.ascontiguousarray(wdst8).astype(bf16),
        "b1p": b1p,
        "W2p": W2p,
        "att_src2": np.ascontiguousarray(
            np.asarray(inputs["att_src2"], np.float32)),
        "att_dst2": np.ascontiguousarray(
            np.asarray(inputs["att_dst2"], np.float32)),
        "b2": np.ascontiguousarray(np.asarray(inputs["b2"], np.float32)),
        "fc_w": np.ascontiguousarray(np.asarray(inputs["fc_w"], np.float32)),
        "fc_b": np.ascontiguousarray(np.asarray(inputs["fc_b"], np.float32)),
    }

    xbf = x.astype(bf16)

    def tilesT(rows_x, ntiles):
        out = np.zeros((ntiles * P, IN), dtype=bf16)
        out[: len(rows_x)] = rows_x
        return np.ascontiguousarray(
            out.reshape(ntiles, P, IN).transpose(2, 0, 1))

    in_maps = []
    for c in range(NC):
        lo_ref, hi_ref = refs[c]
        xref = np.concatenate([
            tilesT(xbf[inv[lo_ref]], NTRlo),
            tilesT(xbf[inv[hi_ref]], NTRhi)], axis=1)
        m = dict(common)
        m["x_refT"] = xref
        m["x_ownT"] = tilesT(xbf[c * NPC:(c + 1) * NPC], NT)
        m["idx16a"] = np.ascontiguousarray(idx16a[c])
        m["idx16b"] = np.ascontiguousarray(idx16b[c])
        m["dstslot"] = np.ascontiguousarray(dstslot[c])
        in_maps.append(m)
    TI = dstslot.shape[2] * P
    return in_maps, sched, TI, NTRlo, NTRhi


def kernel(**inputs) -> np.ndarray:
    from concourse import bass_utils

    cfg = dict(CFG)
    in_maps, sched, TI, NTRlo, NTRhi = build_in_maps(inputs, cfg)
    nc = build_kernel(cfg, sched, TI, NTRlo, NTRhi)
    res = bass_utils.run_bass_kernel_spmd(
        nc, in_maps, core_ids=list(range(cfg["NCORES"])),
        trace=bool(int(os.environ.get("GAT_TRACE", "0"))))
    kernel.last_results = res
    out = np.concatenate([r["out"] for r in res.results], axis=0)
    return out.astype(np.float32)
